# revision 1
# baseline (speedup 1.0000x reference)
"""Distributed Trainium2 kernel for nn_Attention_72722386256499.

Full inputs in, full output out.  Internally shards the 32 (B,H)
attention problems over 8 NeuronCores: core m handles batch m//2,
heads [4*(m%2), 4*(m%2)+4).  The small 1x1-conv weights are sliced and
replicated host-side; the output projection is computed as per-core
partial products summed on the host (data-parallel reduce in unshard).
"""

import sys

sys.path.insert(0, "/opt/trn_rl_repo")

import ml_dtypes
import numpy as np

import bass_rust
import concourse.bass as bass
import concourse.mybir as mybir
import concourse.tile as tile
from concourse import masks
from concourse.bass_utils import run_bass_kernel_spmd

B, C, L = 4, 512, 2048
H, D = 8, 64
HPC = 4  # heads per core
NCORES = 8
FP = mybir.dt.float32

# Matmul compute dtype: float32 (4 cyc/row) or float32r (1 cyc/row @ N>=256).
MM_DT = mybir.dt.bfloat16

TRACE_MODE = False
LAST_RESULT = None
_NC_CACHE = {}


def _split_waits(nc, max_waits=1):
    """walrus here rejects >1 sync wait per instruction; hoist extras onto
    single-wait NoOps just before the instruction on the same engine."""
    counter = 0
    for f in nc.m.functions:
        for bb in f.blocks:
            il = bb.instructions
            new_list = []
            changed = False
            for inst in il:
                si = inst.sync_info
                if si is None:
                    new_list.append(inst)
                    continue
                waits = list(si.on_wait)
                if len(waits) > max_waits:
                    keep = waits[-max_waits:]
                    for w in waits[:-max_waits]:
                        counter += 1
                        nop = mybir.InstNoOp(
                            name=f"I-waitsplit-{counter}", ins=[], outs=[]
                        )
                        nop.engine = inst.engine
                        nop.sync_info = bass_rust.SyncInfo(on_wait=[w], on_update=[])
                        new_list.append(nop)
                        nc.register_instruction(nop, overwrite=True)
                    inst.sync_info = bass_rust.SyncInfo(
                        on_wait=keep, on_update=list(si.on_update)
                    )
                    changed = True
                new_list.append(inst)
            if changed:
                il.clear()
                il.extend(new_list)
    return counter


def _mm(nc, out, lhsT, rhs, start, stop):
    nc.tensor.matmul(out, lhsT, rhs, start=start, stop=stop)


def build_nc():
    nc = bass.Bass()
    x_ext = nc.declare_dram_parameter("x", [C, L], MM_DT, isOutput=False)
    wq_ext = nc.declare_dram_parameter("wq", [C, HPC * D], MM_DT, isOutput=False)
    wk_ext = nc.declare_dram_parameter("wk", [C, HPC * D], MM_DT, isOutput=False)
    wv_ext = nc.declare_dram_parameter("wv", [C, HPC * D], MM_DT, isOutput=False)
    wo_ext = nc.declare_dram_parameter("wo", [HPC * D, C], MM_DT, isOutput=False)
    out_ext = nc.declare_dram_parameter("out", [C, L], MM_DT, isOutput=True)
    scratch = nc.dram_tensor("scratch", [HPC, L, D], MM_DT)

    NJ = L // 128  # 16 j tiles per head
    NIB = L // 512  # 4 i blocks per head
    NCC = C // 128  # 4 contraction chunks

    with tile.TileContext(nc) as tc:
        with (
            tc.tile_pool(name="const", bufs=1) as cpool,
            tc.tile_pool(name="exp", bufs=4) as epool,
            tc.tile_pool(name="o2", bufs=6) as o2pool,
            tc.tile_pool(name="rz", bufs=8) as rzpool,
            tc.tile_pool(name="fout", bufs=3) as fpool,
            tc.tile_pool(name="ps", bufs=2, space="PSUM") as ppool,
        ):
            # ---- persistent SBUF tensors ----
            # MDT tensors feed TensorE matmuls; float32r must be rounded
            # at the producing instruction (DMA/copy/activation output).
            MDT = MM_DT
            x_sbs = [
                [
                    cpool.tile(
                        [128, 512], MDT, tag=f"x{ci}_{lb}", name=f"x_sb{ci}_{lb}"
                    )
                    for lb in range(NIB)
                ]
                for ci in range(NCC)
            ]
            wq_sb = cpool.tile([128, NCC, HPC * D], MDT, tag="wq")
            wk_sb = cpool.tile([128, NCC, HPC * D], MDT, tag="wk")
            wv_sb = cpool.tile([128, NCC, HPC * D], MDT, tag="wv")
            wo_sb = cpool.tile([128, 2, C], MDT, tag="wo")
            q_sbs = [cpool.tile([128, L], MDT, tag=f"q{g}", name=f"q_sb{g}") for g in range(2)]
            k_sbs = [cpool.tile([128, L], MDT, tag=f"k{g}", name=f"k_sb{g}") for g in range(2)]
            # vT1 split into 4 j-quarters so PV can start before all 16 done
            vT1s = [
                cpool.tile([128, 4, HPC, D + 1], MDT, tag=f"vT1{qt}", name=f"vT1_sb{qt}")
                for qt in range(4)
            ]
            out2_sb = cpool.tile([128, HPC, NJ, D], MDT, tag="out2")
            outrs_sbs = [cpool.tile([128, L], MDT, tag=f"outrs{g}", name=f"outrs_sb{g}") for g in range(2)]
            ident = cpool.tile([128, 128], MDT, tag="ident")

            # input DMAs first so transfers start during the preamble;
            # lb-major order so block 0's columns land first
            nc.sync.dma_start(
                out=wq_sb, in_=wq_ext.rearrange("(ci p) n -> p ci n", p=128)
            )
            nc.sync.dma_start(
                out=wk_sb, in_=wk_ext.rearrange("(ci p) n -> p ci n", p=128)
            )
            for lb in range(NIB):
                for ci in range(NCC):
                    nc.sync.dma_start(
                        out=x_sbs[ci][lb],
                        in_=x_ext[
                            ci * 128 : (ci + 1) * 128, lb * 512 : (lb + 1) * 512
                        ],
                    )
            nc.sync.dma_start(
                out=wv_sb, in_=wv_ext.rearrange("(ci p) n -> p ci n", p=128)
            )
            nc.sync.dma_start(
                out=wo_sb, in_=wo_ext.rearrange("(rc p) o -> p rc o", p=128)
            )

            masks.make_identity(nc, ident[:, :])
            ones_f32 = cpool.tile([128, 4 * HPC], FP, tag="ones")
            nc.vector.memset(ones_f32, 1.0)
            for qt in range(4):
                nc.vector.tensor_copy(
                    out=vT1s[qt][:, :, :, D : D + 1],
                    in_=ones_f32.rearrange("p (a b) -> p a b", b=HPC).unsqueeze(-1),
                )

            # ---- projection emitters (used pre-B and as in-block tasks) ----
            def qk_group(w_sb, g, dst, lb, eng="vector", ptag="s"):
                def t():
                    ps = ppool.tile([128, 512], FP, tag=ptag)
                    for ci in range(NCC):
                        _mm(
                            nc,
                            ps,
                            w_sb[:, ci, g * 128 : (g + 1) * 128],
                            x_sbs[ci][lb][:, :],
                            start=(ci == 0),
                            stop=(ci == NCC - 1),
                        )
                    dsl = dst[:, lb * 512 : (lb + 1) * 512]
                    if eng == "scalar":
                        nc.scalar.copy(out=dsl, in_=ps)
                    else:
                        nc.vector.tensor_copy(out=dsl, in_=ps)
                return t

            def v_quarter(qt, ptag="s"):
                def t():
                    for j4 in range(4):
                        jt = qt * 4 + j4
                        ps = ppool.tile([128, HPC * D], FP, tag=ptag)
                        for ci in range(NCC):
                            _mm(
                                nc,
                                ps,
                                x_sbs[ci][jt // 4][
                                    :, (jt % 4) * 128 : (jt % 4 + 1) * 128
                                ],
                                wv_sb[:, ci, :],
                                start=(ci == 0),
                                stop=(ci == NCC - 1),
                            )
                        nc.vector.tensor_copy(
                            out=vT1s[qt][:, j4, :, 0:D],
                            in_=ps.rearrange("p (h d) -> p h d", h=HPC),
                        )
                return t

            # minimal pre-B work: enough q/k/vT for block 0's first half
            qk_group(wq_sb, 0, q_sbs[0], 0, "scalar")()
            qk_group(wk_sb, 0, k_sbs[0], 0, "vector")()
            qk_group(wq_sb, 0, q_sbs[0], 1, "scalar")()
            qk_group(wk_sb, 0, k_sbs[0], 1, "vector")()
            v_quarter(0)()
            v_quarter(1)()

            # per-block deferred tasks: {jt: emitter}.  Needed-by deadlines:
            # k0lb2/vTq2 by jt8 of block 0, k0lb3/vTq3 by jt12 of block 0,
            # q0lbN by block N start, q1/k1 by block 4 start.
            block_tasks = {
                0: {3: qk_group(wk_sb, 0, k_sbs[0], 2, ptag="t"),
                    6: v_quarter(2, ptag="t"),
                    9: qk_group(wk_sb, 0, k_sbs[0], 3, ptag="t"),
                    12: v_quarter(3, ptag="t")},
                1: {0: qk_group(wq_sb, 0, q_sbs[0], 2, ptag="t"),
                    4: qk_group(wq_sb, 1, q_sbs[1], 0, ptag="t"),
                    8: qk_group(wq_sb, 1, q_sbs[1], 1, ptag="t"),
                    12: qk_group(wq_sb, 0, q_sbs[0], 3, ptag="t")},
                2: {0: qk_group(wq_sb, 1, q_sbs[1], 2, ptag="t"),
                    4: qk_group(wq_sb, 1, q_sbs[1], 3, ptag="t"),
                    8: qk_group(wk_sb, 1, k_sbs[1], 0, ptag="t"),
                    12: qk_group(wk_sb, 1, k_sbs[1], 1, ptag="t")},
                3: {0: qk_group(wk_sb, 1, k_sbs[1], 2, ptag="t"),
                    4: qk_group(wk_sb, 1, k_sbs[1], 3, ptag="t")},
                7: {15: "outrs1_early"},
            }

            # ---- phase B/C: attention per (head-pair, i-block) ----
            # Two heads of a pair sit on partition halves 0:64 / 64:128, so
            # their K=64 S^T matmuls row-tile into disjoint PE quadrants and
            # run concurrently.  Each block's transpose/normalize tail is
            # split into 2-transpose chunks spread over the NEXT block so the
            # PE never runs >2 consecutive non-S^T ops while ACT needs food.
            scratch_rs = scratch.ap().flatten().rearrange("(q e) -> q e", e=L)
            pending = []

            def make_flush_unit(h, ib2, cc, o2T):
                def u():
                    tg = ib2 * 4 + cc
                    ps_t = ppool.tile([128, 512], MDT, tag="t")
                    nc.tensor.transpose(
                        ps_t[:, 0:128],
                        o2T[:, cc * 128 : (cc + 1) * 128],
                        ident[:, :],
                    )
                    rz = rzpool.tile([128, 1], FP, tag="rz")
                    nc.vector.reciprocal(out=rz, in_=ps_t[:, D : D + 1])
                    nc.vector.tensor_scalar_mul(
                        out=out2_sb[:, h, tg, :],
                        in0=ps_t[:, 0:D],
                        scalar1=rz,
                    )
                    if h >= 2 and ib2 == NIB - 1:
                        # final block: latency-critical -> contiguous 16KB
                        # write per unit, spread across DMA queues/engines
                        weng = nc.sync if cc % 2 == 0 else nc.gpsimd
                        weng.dma_start(
                            out=scratch[h, tg * 128 : (tg + 1) * 128, :],
                            in_=out2_sb[:, h, tg, :],
                        )
                    elif cc == 3:
                        nc.sync.dma_start(
                            out=scratch[
                                h, ib2 * 512 : (ib2 + 1) * 512, :
                            ].rearrange("(c2 p) d -> p c2 d", p=128),
                            in_=out2_sb[:, h, ib2 * 4 : (ib2 + 1) * 4, :],
                        )
                    if cc == 3 and h == 1 and ib2 == NIB - 1:
                        nc.sync.dma_start(
                            out=outrs_sbs[0], in_=scratch_rs[0:128, :]
                        )
                return u

            FLUSH_JTS = (2, 6, 10, 14)
            for g in range(2):
                for ib in range(NIB):
                    bi = g * NIB + ib
                    tasks = block_tasks.get(bi, {})
                    i0 = ib * 512
                    ps_oA = ppool.tile([128, 512], FP, tag="o")
                    ps_oB = ppool.tile([128, 512], FP, tag="o")
                    for jt in range(NJ):
                        if jt in FLUSH_JTS:
                            for _ in range(2):
                                if pending:
                                    pending.pop(0)()
                        if jt in tasks:
                            t = tasks[jt]
                            if t == "outrs1_early":
                                nc.sync.dma_start(
                                    out=outrs_sbs[1][0:48, :],
                                    in_=scratch_rs[128:176, :],
                                )
                                nc.gpsimd.dma_start(
                                    out=outrs_sbs[1][64:112, :],
                                    in_=scratch_rs[192:240, :],
                                )
                            else:
                                t()
                        ps_s = ppool.tile([128, 1024], FP, tag="s")
                        for hp in range(2):
                            p0 = hp * 64
                            _mm(
                                nc,
                                ps_s[:, hp * 512 : (hp + 1) * 512],
                                k_sbs[g][p0 : p0 + 64, jt * 128 : (jt + 1) * 128],
                                q_sbs[g][p0 : p0 + 64, i0 : i0 + 512],
                                start=True,
                                stop=True,
                            )
                        ex = epool.tile([128, 1024], MDT, tag="exp")
                        nc.scalar.activation(
                            out=ex, in_=ps_s, func=mybir.ActivationFunctionType.Exp
                        )
                        for hp, ps_o in ((0, ps_oA), (1, ps_oB)):
                            _mm(
                                nc,
                                ps_o[0 : D + 1, :],
                                vT1s[jt // 4][:, jt % 4, 2 * g + hp, :],
                                ex[:, hp * 512 : (hp + 1) * 512],
                                start=(jt == 0),
                                stop=(jt == NJ - 1),
                            )
                    # copy accumulators out promptly to free the PSUM banks
                    for hp, ps_o in ((0, ps_oA), (1, ps_oB)):
                        h = 2 * g + hp
                        o2T = o2pool.tile([128, 512], MDT, tag="o2T")
                        nc.vector.tensor_copy(
                            out=o2T[0 : D + 1, :], in_=ps_o[0 : D + 1, :]
                        )
                        for cc in range(4):
                            pending.append(make_flush_unit(h, ib, cc, o2T))
            while pending:
                pending.pop(0)()
            # late outrs1 rows produced by the final block (r in [48,64) of
            # each head); the rest was loaded early inside block 7
            nc.sync.dma_start(
                out=outrs_sbs[1][48:64, :], in_=scratch_rs[176:192, :]
            )
            nc.gpsimd.dma_start(
                out=outrs_sbs[1][112:128, :], in_=scratch_rs[240:256, :]
            )
            # keep PE warm while the last scratch writes/outrs reads land:
            # first on the identity, then on the early outrs rows (which
            # forces these to run after that DMA, covering the late wait)
            warm2 = ppool.tile([128, 512], FP, tag="o")
            for _ in range(10):
                nc.tensor.matmul(
                    warm2[:, 0:128], ident[:, :], ident[:, :], start=True, stop=True
                )
            for r in range(10):
                nc.tensor.matmul(
                    warm2,
                    wo_sb[0:48, 1, 0:128],
                    outrs_sbs[1][0:48, 0:512],
                    start=True,
                    stop=True,
                )

            # ---- phase D: output projection on the reshaped rows ----
            # copies alternate DVE/ACT; one batched output DMA per og row
            # block, alternating sync/gpsimd queues to parallelize enqueue.
            for og in range(4):
                fo = fpool.tile([128, L], MDT, tag="fout")
                for lb in range(NIB):
                    it = og * NIB + lb
                    ps_f = ppool.tile(
                        [128, 512], FP, tag="o" if it % 2 == 0 else "t"
                    )
                    for rc in range(2):
                        _mm(
                            nc,
                            ps_f,
                            wo_sb[:, rc, og * 128 : (og + 1) * 128],
                            outrs_sbs[rc][:, lb * 512 : (lb + 1) * 512],
                            start=(rc == 0),
                            stop=(rc == 1),
                        )
                    fsl = fo[:, lb * 512 : (lb + 1) * 512]
                    if it % 2 == 0:
                        nc.vector.tensor_copy(out=fsl, in_=ps_f)
                    else:
                        nc.scalar.copy(out=fsl, in_=ps_f)
                eng = nc.sync if og % 2 == 0 else nc.gpsimd
                eng.dma_start(
                    out=out_ext[og * 128 : (og + 1) * 128, :], in_=fo
                )

    _split_waits(nc)
    return nc


def _get_nc():
    key = str(MM_DT)
    if key not in _NC_CACHE:
        _NC_CACHE[key] = build_nc()
    return _NC_CACHE[key]


def kernel(x, w_qkv, w_out, b_out):
    global LAST_RESULT
    x = np.asarray(x, dtype=np.float32)
    w_qkv = np.asarray(w_qkv, dtype=np.float32)
    w_out = np.asarray(w_out, dtype=np.float32)
    b_out = np.asarray(b_out, dtype=np.float32)

    scale = D**-0.5
    in_maps = []
    for m in range(NCORES):
        b = m // 2
        hs = [4 * (m % 2) + i for i in range(HPC)]
        q_rows = np.concatenate([np.arange(h * D, (h + 1) * D) for h in hs])
        wq = np.ascontiguousarray((w_qkv[q_rows, :] * scale).T)
        wk = np.ascontiguousarray(w_qkv[C + q_rows, :].T)
        wv = np.ascontiguousarray(w_qkv[2 * C + q_rows, :].T)
        wo = np.ascontiguousarray(w_out[:, q_rows].T)
        bf16 = ml_dtypes.bfloat16
        in_maps.append(
            {
                "x": np.ascontiguousarray(x[b]).astype(bf16),
                "wq": wq.astype(bf16),
                "wk": wk.astype(bf16),
                "wv": wv.astype(bf16),
                "wo": wo.astype(bf16),
            }
        )

    nc = _get_nc()
    res = run_bass_kernel_spmd(
        nc, in_maps, core_ids=list(range(NCORES)), trace=TRACE_MODE
    )
    LAST_RESULT = res

    out = np.empty((B, C, L), dtype=np.float32)
    for b in range(B):
        out[b] = res.results[2 * b]["out"].astype(np.float32) + res.results[
            2 * b + 1
        ]["out"].astype(np.float32)
        out[b] += b_out[:, None]
    return out



# revision 4
# speedup vs baseline: 1.2110x; 1.2110x over previous
"""Distributed Trainium2 kernel for nn_Attention_72722386256499.

Full inputs in, full output out.  Internally shards the 32 (B,H)
attention problems over 8 NeuronCores: core m handles batch m//2,
heads [4*(m%2), 4*(m%2)+4).  Weights are sliced and replicated
host-side; the output projection is computed as per-core partial
products summed on the host.

v2: host-prepacked contiguous DMA layouts (fast preamble), per-j-tile
V-projection subtasks scheduled just-in-time, gapless tail (pipelined
output projection over six PSUM slots), block-boundary degapping.
"""

import sys

sys.path.insert(0, "/opt/trn_rl_repo")

import ml_dtypes
import numpy as np

import bass_rust
import concourse.bass as bass
import concourse.mybir as mybir
import concourse.tile as tile
from concourse import masks
from concourse.bass_utils import run_bass_kernel_spmd

B, C, L = 4, 512, 2048
H, D = 8, 64
HPC = 4  # heads per core
NCORES = 8
FP = mybir.dt.float32

MM_DT = mybir.dt.bfloat16

NJ = L // 128  # 16 j tiles
NIB = L // 512  # 4 i blocks
NCC = C // 128  # 4 contraction chunks

TRACE_MODE = False
LAST_RESULT = None
_NC_CACHE = {}


def _split_waits(nc, max_waits=1):
    """walrus here rejects >1 sync wait per instruction; hoist extras onto
    single-wait NoOps just before the instruction on the same engine."""
    counter = 0
    for f in nc.m.functions:
        for bb in f.blocks:
            il = bb.instructions
            new_list = []
            changed = False
            for inst in il:
                si = inst.sync_info
                if si is None:
                    new_list.append(inst)
                    continue
                waits = list(si.on_wait)
                if len(waits) > max_waits:
                    keep = waits[-max_waits:]
                    for w in waits[:-max_waits]:
                        counter += 1
                        nop = mybir.InstNoOp(
                            name=f"I-waitsplit-{counter}", ins=[], outs=[]
                        )
                        nop.engine = inst.engine
                        nop.sync_info = bass_rust.SyncInfo(on_wait=[w], on_update=[])
                        new_list.append(nop)
                        nc.register_instruction(nop, overwrite=True)
                    inst.sync_info = bass_rust.SyncInfo(
                        on_wait=keep, on_update=list(si.on_update)
                    )
                    changed = True
                new_list.append(inst)
            if changed:
                il.clear()
                il.extend(new_list)
    return counter


def build_nc():
    nc = bass.Bass()
    # Host-prepacked layouts: every DRAM line is contiguous per partition.
    x_ext = nc.declare_dram_parameter("x", [128, NIB, NCC, 512], MM_DT, isOutput=False)
    wq_ext = nc.declare_dram_parameter("wq", [128, NCC, HPC * D], MM_DT, isOutput=False)
    wk_ext = nc.declare_dram_parameter("wk", [128, NCC, HPC * D], MM_DT, isOutput=False)
    wv_ext = nc.declare_dram_parameter("wv", [128, NCC, HPC * D], MM_DT, isOutput=False)
    wo_ext = nc.declare_dram_parameter("wo", [128, 2, C], MM_DT, isOutput=False)
    out_ext = nc.declare_dram_parameter("out", [C, L], MM_DT, isOutput=True)
    scratch = nc.dram_tensor("scratch", [HPC, L, D], MM_DT)

    with tile.TileContext(nc) as tc:
        with (
            tc.tile_pool(name="const", bufs=1) as cpool,
            tc.tile_pool(name="exp", bufs=4) as epool,
            tc.tile_pool(name="o2", bufs=6) as o2pool,
            tc.tile_pool(name="rz", bufs=8) as rzpool,
            tc.tile_pool(name="fout", bufs=4) as fpool,
            tc.tile_pool(name="ps", bufs=2, space="PSUM") as ppool,
        ):
            # ---- persistent SBUF tensors ----
            MDT = MM_DT
            x_sb = cpool.tile([128, NIB, NCC, 512], MDT, tag="x")
            wq_sb = cpool.tile([128, NCC, HPC * D], MDT, tag="wq")
            wk_sb = cpool.tile([128, NCC, HPC * D], MDT, tag="wk")
            wv_sb = cpool.tile([128, NCC, HPC * D], MDT, tag="wv")
            wo_sb = cpool.tile([128, 2, C], MDT, tag="wo")
            q_sbs = [
                cpool.tile([128, L], MDT, tag=f"q{g}", name=f"q_sb{g}")
                for g in range(2)
            ]
            k_sbs = [
                cpool.tile([128, L], MDT, tag=f"k{g}", name=f"k_sb{g}")
                for g in range(2)
            ]
            vT1s = [
                cpool.tile([128, 4, HPC, D + 1], MDT, tag=f"vT1{qt}", name=f"vT1_sb{qt}")
                for qt in range(4)
            ]
            out2_sb = cpool.tile([128, HPC, NJ, D], MDT, tag="out2")
            outrs_sbs = [
                cpool.tile([128, L], MDT, tag=f"outrs{g}", name=f"outrs_sb{g}")
                for g in range(2)
            ]
            ident = cpool.tile([128, 128], MDT, tag="ident")

            # ---- input DMAs, critical-first; all contiguous lines ----
            nc.sync.dma_start(out=wq_sb, in_=wq_ext.ap())
            nc.sync.dma_start(out=wk_sb, in_=wk_ext.ap())
            nc.sync.dma_start(out=x_sb[:, 0], in_=x_ext[:, 0])
            nc.sync.dma_start(out=wv_sb, in_=wv_ext.ap())
            for lb in range(1, NIB):
                nc.sync.dma_start(out=x_sb[:, lb], in_=x_ext[:, lb])
            nc.sync.dma_start(out=wo_sb, in_=wo_ext.ap())

            masks.make_identity(nc, ident[:, :])
            ones_f32 = cpool.tile([128, 4 * HPC], FP, tag="ones")
            nc.vector.memset(ones_f32, 1.0)
            for qt in range(4):
                nc.vector.tensor_copy(
                    out=vT1s[qt][:, :, :, D : D + 1],
                    in_=ones_f32.rearrange("p (a b) -> p a b", b=HPC).unsqueeze(-1),
                )

            # ---- projection task emitters ----
            def qk_group(w_sb, g, dst, lb, eng="vector", ptag="t"):
                def t():
                    ps = ppool.tile([128, 512], FP, tag=ptag)
                    for ci in range(NCC):
                        nc.tensor.matmul(
                            ps,
                            w_sb[:, ci, g * 128 : (g + 1) * 128],
                            x_sb[:, lb, ci, :],
                            start=(ci == 0),
                            stop=(ci == NCC - 1),
                        )
                    dsl = dst[:, lb * 512 : (lb + 1) * 512]
                    if eng == "scalar":
                        nc.scalar.copy(out=dsl, in_=ps)
                    else:
                        nc.vector.tensor_copy(out=dsl, in_=ps)
                return t

            def v_sub(j, ptag="t"):
                # vT1 for j-tile j (all 4 heads)
                def t():
                    qt, j4 = j // 4, j % 4
                    ps = ppool.tile([128, HPC * D], FP, tag=ptag)
                    for ci in range(NCC):
                        nc.tensor.matmul(
                            ps,
                            x_sb[:, qt, ci, j4 * 128 : (j4 + 1) * 128],
                            wv_sb[:, ci, :],
                            start=(ci == 0),
                            stop=(ci == NCC - 1),
                        )
                    nc.vector.tensor_copy(
                        out=vT1s[qt][:, j4, :, 0:D],
                        in_=ps.rearrange("p (h d) -> p h d", h=HPC),
                    )
                return t

            # minimal prework for block 0's start
            qk_group(wq_sb, 0, q_sbs[0], 0, "scalar", ptag="s")()
            qk_group(wk_sb, 0, k_sbs[0], 0, "vector", ptag="s")()
            v_sub(0)()

            # deferred tasks: slot u -> emitters (run just before QK(u))
            tasks = {}

            def add_task(u, t):
                tasks.setdefault(u, []).append(t)

            for j in range(2, 16):
                add_task(j - 2, v_sub(j))
            add_task(0, v_sub(1))
            add_task(1, qk_group(wk_sb, 0, k_sbs[0], 1))
            add_task(5, qk_group(wk_sb, 0, k_sbs[0], 2))
            add_task(9, qk_group(wk_sb, 0, k_sbs[0], 3))
            add_task(14, qk_group(wq_sb, 0, q_sbs[0], 1))
            add_task(20, qk_group(wq_sb, 0, q_sbs[0], 2))
            add_task(36, qk_group(wq_sb, 0, q_sbs[0], 3))
            add_task(46, qk_group(wk_sb, 1, k_sbs[1], 0))
            add_task(50, qk_group(wk_sb, 1, k_sbs[1], 1))
            add_task(54, qk_group(wk_sb, 1, k_sbs[1], 2))
            add_task(58, qk_group(wk_sb, 1, k_sbs[1], 3))
            add_task(60, qk_group(wq_sb, 1, q_sbs[1], 0))
            add_task(74, qk_group(wq_sb, 1, q_sbs[1], 1))
            add_task(90, qk_group(wq_sb, 1, q_sbs[1], 2))
            add_task(106, qk_group(wq_sb, 1, q_sbs[1], 3))

            # ---- attention main loop ----
            scratch_rs = scratch.ap().flatten().rearrange("(q e) -> q e", e=L)
            pending = []

            def make_flush_unit(h, ib2, cc, o2T):
                def u():
                    tg = ib2 * 4 + cc
                    ps_t = ppool.tile([128, 512], MDT, tag="t")
                    nc.tensor.transpose(
                        ps_t[:, 0:128],
                        o2T[:, cc * 128 : (cc + 1) * 128],
                        ident[:, :],
                    )
                    rz = rzpool.tile([128, 1], FP, tag="rz")
                    nc.vector.reciprocal(out=rz, in_=ps_t[:, D : D + 1])
                    nc.vector.tensor_scalar_mul(
                        out=out2_sb[:, h, tg, :],
                        in0=ps_t[:, 0:D],
                        scalar1=rz,
                    )
                    if h >= 2 and ib2 == NIB - 1:
                        # final block: latency-critical -> contiguous 16KB
                        # write per unit, spread across DMA queues/engines
                        weng = nc.sync if cc % 2 == 0 else nc.gpsimd
                        weng.dma_start(
                            out=scratch[h, tg * 128 : (tg + 1) * 128, :],
                            in_=out2_sb[:, h, tg, :],
                        )
                    elif cc == 3:
                        nc.sync.dma_start(
                            out=scratch[
                                h, ib2 * 512 : (ib2 + 1) * 512, :
                            ].rearrange("(c2 p) d -> p c2 d", p=128),
                            in_=out2_sb[:, h, ib2 * 4 : (ib2 + 1) * 4, :],
                        )
                    if cc == 3 and h == 1 and ib2 == NIB - 1:
                        nc.sync.dma_start(
                            out=outrs_sbs[0], in_=scratch_rs[0:128, :]
                        )
                return u

            DRAIN_JTS = (2, 4, 6, 8, 10, 12, 14, 15)
            o_tiles = None
            for u in range(128):
                g, ib, jt = u // 64, (u // 16) % 4, u % 16
                i0 = ib * 512
                if jt == 0:
                    ps_oA = ppool.tile([128, 512], FP, tag="o")
                    ps_oB = ppool.tile([128, 512], FP, tag="o")
                    o_tiles = (ps_oA, ps_oB)
                if jt in DRAIN_JTS and pending:
                    pending.pop(0)()
                if u == 127:
                    # early outrs1 rows (from blocks 4-6 flushes, already in
                    # scratch): 0:48 of each g1 head group
                    nc.sync.dma_start(
                        out=outrs_sbs[1][0:48, :], in_=scratch_rs[128:176, :]
                    )
                    nc.gpsimd.dma_start(
                        out=outrs_sbs[1][64:112, :], in_=scratch_rs[192:240, :]
                    )
                for t in tasks.get(u, ()):
                    t()
                ps_s = ppool.tile([128, 1024], FP, tag="s")
                for hp in range(2):
                    p0 = hp * 64
                    nc.tensor.matmul(
                        ps_s[:, hp * 512 : (hp + 1) * 512],
                        k_sbs[g][p0 : p0 + 64, jt * 128 : (jt + 1) * 128],
                        q_sbs[g][p0 : p0 + 64, i0 : i0 + 512],
                        start=True,
                        stop=True,
                    )
                ex = epool.tile([128, 1024], MDT, tag="exp")
                nc.scalar.activation(
                    out=ex, in_=ps_s, func=mybir.ActivationFunctionType.Exp
                )
                for hp in range(2):
                    nc.tensor.matmul(
                        o_tiles[hp][0 : D + 1, :],
                        vT1s[jt // 4][:, jt % 4, 2 * g + hp, :],
                        ex[:, hp * 512 : (hp + 1) * 512],
                        start=(jt == 0),
                        stop=(jt == NJ - 1),
                    )
                if jt == NJ - 1:
                    # free the PSUM accumulators promptly, then queue flush
                    for hp in range(2):
                        h = 2 * g + hp
                        o2T = o2pool.tile([128, 512], MDT, tag="o2T")
                        nc.vector.tensor_copy(
                            out=o2T[0 : D + 1, :], in_=o_tiles[hp][0 : D + 1, :]
                        )
                        for cc in range(4):
                            pending.append(make_flush_unit(h, ib, cc, o2T))

            # ---- tail: drain last flushes interleaved with og0 rc0 MMs ----
            psA = ppool.tile([128, 1024], FP, tag="s")
            psB = ppool.tile([128, 1024], FP, tag="s")
            ps_slots = [
                psA[:, 0:512],
                psA[:, 512:1024],
                psB[:, 0:512],
                psB[:, 512:1024],
                None,  # filled with 't' tiles after the flush drain
                None,
            ]

            def phd_mm(it, rc, start, stop):
                og, lb = it // NIB, it % NIB
                nc.tensor.matmul(
                    ps_slots[it % 6],
                    wo_sb[:, rc, og * 128 : (og + 1) * 128],
                    outrs_sbs[rc][:, lb * 512 : (lb + 1) * 512],
                    start=start,
                    stop=stop,
                )

            # og0 rc0 interleaved with the last block's flush units
            fl_i = 0
            for it in range(NIB):  # og0, lb 0..3
                while fl_i < 2 * (it + 1) and pending:
                    pending.pop(0)()
                    fl_i += 1
                phd_mm(it, 0, start=True, stop=False)
            while pending:
                pending.pop(0)()
            # safe to claim the 't' rotation now that flushes are all emitted
            pst0 = ppool.tile([128, 512], FP, tag="t")
            pst1 = ppool.tile([128, 512], FP, tag="t")
            ps_slots[4] = pst0
            ps_slots[5] = pst1
            # late outrs1 rows from the final block
            nc.sync.dma_start(
                out=outrs_sbs[1][48:64, :], in_=scratch_rs[176:192, :]
            )
            nc.gpsimd.dma_start(
                out=outrs_sbs[1][112:128, :], in_=scratch_rs[240:256, :]
            )
            # finish og0 (rc1) then stream og1..og3; copies alternate engines
            fouts = [
                fpool.tile([128, L], MDT, tag="fout", name=f"fo{og}")
                for og in range(4)
            ]
            for it in range(NIB):
                phd_mm(it, 1, start=False, stop=True)
                fsl = fouts[0][:, (it % NIB) * 512 : (it % NIB + 1) * 512]
                if it % 2 == 0:
                    nc.vector.tensor_copy(out=fsl, in_=ps_slots[it % 6])
                else:
                    nc.scalar.copy(out=fsl, in_=ps_slots[it % 6])
            nc.sync.dma_start(out=out_ext[0:128, :], in_=fouts[0])
            for it in range(NIB, 4 * NIB):
                og, lb = it // NIB, it % NIB
                phd_mm(it, 0, start=True, stop=False)
                phd_mm(it, 1, start=False, stop=True)
                fsl = fouts[og][:, lb * 512 : (lb + 1) * 512]
                if it % 2 == 0:
                    nc.vector.tensor_copy(out=fsl, in_=ps_slots[it % 6])
                else:
                    nc.scalar.copy(out=fsl, in_=ps_slots[it % 6])
                if lb == NIB - 1:
                    eng = nc.gpsimd if og % 2 == 0 else nc.sync
                    eng.dma_start(
                        out=out_ext[og * 128 : (og + 1) * 128, :], in_=fouts[og]
                    )

    _split_waits(nc)
    return nc


def _get_nc():
    key = str(MM_DT)
    if key not in _NC_CACHE:
        _NC_CACHE[key] = build_nc()
    return _NC_CACHE[key]


def _prepack_x(xb):
    # x[b] (C, L) -> [p, lb, ci, n] with c = ci*128+p, l = lb*512+n
    return np.ascontiguousarray(
        xb.reshape(NCC, 128, NIB, 512).transpose(1, 2, 0, 3)
    )


def _prepack_w(w):
    # (C, 256) -> [p, ci, n]
    return np.ascontiguousarray(w.reshape(NCC, 128, HPC * D).transpose(1, 0, 2))


def _prepack_wo(w):
    # (256, C) -> [p, rc, o]
    return np.ascontiguousarray(w.reshape(2, 128, C).transpose(1, 0, 2))


def kernel(x, w_qkv, w_out, b_out):
    global LAST_RESULT
    x = np.asarray(x, dtype=np.float32)
    w_qkv = np.asarray(w_qkv, dtype=np.float32)
    w_out = np.asarray(w_out, dtype=np.float32)
    b_out = np.asarray(b_out, dtype=np.float32)

    scale = D**-0.5
    bf16 = ml_dtypes.bfloat16
    xs = [_prepack_x(x[b]).astype(bf16) for b in range(B)]
    in_maps = []
    for m in range(NCORES):
        b = m // 2
        hs = [4 * (m % 2) + i for i in range(HPC)]
        q_rows = np.concatenate([np.arange(h * D, (h + 1) * D) for h in hs])
        wq = np.ascontiguousarray((w_qkv[q_rows, :] * scale).T)
        wk = np.ascontiguousarray(w_qkv[C + q_rows, :].T)
        wv = np.ascontiguousarray(w_qkv[2 * C + q_rows, :].T)
        wo = np.ascontiguousarray(w_out[:, q_rows].T)
        in_maps.append(
            {
                "x": xs[b],
                "wq": _prepack_w(wq).astype(bf16),
                "wk": _prepack_w(wk).astype(bf16),
                "wv": _prepack_w(wv).astype(bf16),
                "wo": _prepack_wo(wo).astype(bf16),
            }
        )

    nc = _get_nc()
    res = run_bass_kernel_spmd(
        nc, in_maps, core_ids=list(range(NCORES)), trace=TRACE_MODE
    )
    LAST_RESULT = res

    out = np.empty((B, C, L), dtype=np.float32)
    for b in range(B):
        out[b] = res.results[2 * b]["out"].astype(np.float32) + res.results[
            2 * b + 1
        ]["out"].astype(np.float32)
        out[b] += b_out[:, None]
    return out


# revision 5
# speedup vs baseline: 1.2127x; 1.0014x over previous
"""Distributed Trainium2 kernel for nn_Attention_72722386256499.

Full inputs in, full output out.  Internally shards the 32 (B,H)
attention problems over 8 NeuronCores: core m handles batch m//2,
heads [4*(m%2), 4*(m%2)+4).  Weights are sliced and replicated
host-side; the output projection is computed as per-core partial
products summed on the host.

v2: host-prepacked contiguous DMA layouts (fast preamble), per-j-tile
V-projection subtasks scheduled just-in-time, gapless tail (pipelined
output projection over six PSUM slots), block-boundary degapping.
"""

import sys

sys.path.insert(0, "/opt/trn_rl_repo")

import ml_dtypes
import numpy as np

import bass_rust
import concourse.bass as bass
import concourse.mybir as mybir
import concourse.tile as tile
from concourse import masks
from concourse.bass_utils import run_bass_kernel_spmd

B, C, L = 4, 512, 2048
H, D = 8, 64
HPC = 4  # heads per core
NCORES = 8
FP = mybir.dt.float32

MM_DT = mybir.dt.bfloat16

NJ = L // 128  # 16 j tiles
NIB = L // 512  # 4 i blocks
NCC = C // 128  # 4 contraction chunks

TRACE_MODE = False
LAST_RESULT = None
_NC_CACHE = {}


def _split_waits(nc, max_waits=1):
    """walrus here rejects >1 sync wait per instruction; hoist extras onto
    single-wait NoOps just before the instruction on the same engine."""
    counter = 0
    for f in nc.m.functions:
        for bb in f.blocks:
            il = bb.instructions
            new_list = []
            changed = False
            for inst in il:
                si = inst.sync_info
                if si is None:
                    new_list.append(inst)
                    continue
                waits = list(si.on_wait)
                if len(waits) > max_waits:
                    keep = waits[-max_waits:]
                    for w in waits[:-max_waits]:
                        counter += 1
                        nop = mybir.InstNoOp(
                            name=f"I-waitsplit-{counter}", ins=[], outs=[]
                        )
                        nop.engine = inst.engine
                        nop.sync_info = bass_rust.SyncInfo(on_wait=[w], on_update=[])
                        new_list.append(nop)
                        nc.register_instruction(nop, overwrite=True)
                    inst.sync_info = bass_rust.SyncInfo(
                        on_wait=keep, on_update=list(si.on_update)
                    )
                    changed = True
                new_list.append(inst)
            if changed:
                il.clear()
                il.extend(new_list)
    return counter


def build_nc():
    nc = bass.Bass()
    # Host-prepacked layouts: every DRAM line is contiguous per partition.
    x_ext = nc.declare_dram_parameter("x", [128, NIB, NCC, 512], MM_DT, isOutput=False)
    wq_ext = nc.declare_dram_parameter("wq", [128, NCC, HPC * D], MM_DT, isOutput=False)
    wk_ext = nc.declare_dram_parameter("wk", [128, NCC, HPC * D], MM_DT, isOutput=False)
    wv_ext = nc.declare_dram_parameter("wv", [128, NCC, HPC * D], MM_DT, isOutput=False)
    wo_ext = nc.declare_dram_parameter("wo", [128, 2, C], MM_DT, isOutput=False)
    out_ext = nc.declare_dram_parameter("out", [C, L], MM_DT, isOutput=True)

    with tile.TileContext(nc) as tc:
        with (
            tc.tile_pool(name="const", bufs=1) as cpool,
            tc.tile_pool(name="exp", bufs=4) as epool,
            tc.tile_pool(name="o2", bufs=6) as o2pool,
            tc.tile_pool(name="rz", bufs=8) as rzpool,
            tc.tile_pool(name="fout", bufs=4) as fpool,
            tc.tile_pool(name="ps", bufs=2, space="PSUM") as ppool,
        ):
            # ---- persistent SBUF tensors ----
            MDT = MM_DT
            x_sb = cpool.tile([128, NIB, NCC, 512], MDT, tag="x")
            wq_sb = cpool.tile([128, NCC, HPC * D], MDT, tag="wq")
            wk_sb = cpool.tile([128, NCC, HPC * D], MDT, tag="wk")
            wv_sb = cpool.tile([128, NCC, HPC * D], MDT, tag="wv")
            wo_sb = cpool.tile([128, 2, C], MDT, tag="wo")
            q_sbs = [
                cpool.tile([128, L], MDT, tag=f"q{g}", name=f"q_sb{g}")
                for g in range(2)
            ]
            k_sbs = [
                cpool.tile([128, L], MDT, tag=f"k{g}", name=f"k_sb{g}")
                for g in range(2)
            ]
            vT1s = [
                cpool.tile([128, 4, HPC, D + 1], MDT, tag=f"vT1{qt}", name=f"vT1_sb{qt}")
                for qt in range(4)
            ]
            out2_sb = cpool.tile([128, HPC, NJ, D], MDT, tag="out2")
            outrs_sbs = [
                cpool.tile([128, L], MDT, tag=f"outrs{g}", name=f"outrs_sb{g}")
                for g in range(2)
            ]
            ident = cpool.tile([128, 128], MDT, tag="ident")

            # ---- input DMAs, critical-first; all contiguous lines ----
            nc.sync.dma_start(out=wq_sb, in_=wq_ext.ap())
            nc.sync.dma_start(out=wk_sb, in_=wk_ext.ap())
            nc.sync.dma_start(out=x_sb[:, 0], in_=x_ext[:, 0])
            nc.sync.dma_start(out=wv_sb, in_=wv_ext.ap())
            for lb in range(1, NIB):
                nc.sync.dma_start(out=x_sb[:, lb], in_=x_ext[:, lb])
            nc.sync.dma_start(out=wo_sb, in_=wo_ext.ap())

            masks.make_identity(nc, ident[:, :])
            ones_f32 = cpool.tile([128, 4 * HPC], FP, tag="ones")
            nc.vector.memset(ones_f32, 1.0)
            for qt in range(4):
                nc.vector.tensor_copy(
                    out=vT1s[qt][:, :, :, D : D + 1],
                    in_=ones_f32.rearrange("p (a b) -> p a b", b=HPC).unsqueeze(-1),
                )

            # ---- projection task emitters ----
            def qk_group(w_sb, g, dst, lb, eng="vector", ptag="t"):
                def t():
                    ps = ppool.tile([128, 512], FP, tag=ptag)
                    for ci in range(NCC):
                        nc.tensor.matmul(
                            ps,
                            w_sb[:, ci, g * 128 : (g + 1) * 128],
                            x_sb[:, lb, ci, :],
                            start=(ci == 0),
                            stop=(ci == NCC - 1),
                        )
                    dsl = dst[:, lb * 512 : (lb + 1) * 512]
                    if eng == "scalar":
                        nc.scalar.copy(out=dsl, in_=ps)
                    else:
                        nc.vector.tensor_copy(out=dsl, in_=ps)
                return t

            def v_sub(j, ptag="t"):
                # vT1 for j-tile j (all 4 heads)
                def t():
                    qt, j4 = j // 4, j % 4
                    ps = ppool.tile([128, HPC * D], FP, tag=ptag)
                    for ci in range(NCC):
                        nc.tensor.matmul(
                            ps,
                            x_sb[:, qt, ci, j4 * 128 : (j4 + 1) * 128],
                            wv_sb[:, ci, :],
                            start=(ci == 0),
                            stop=(ci == NCC - 1),
                        )
                    nc.vector.tensor_copy(
                        out=vT1s[qt][:, j4, :, 0:D],
                        in_=ps.rearrange("p (h d) -> p h d", h=HPC),
                    )
                return t

            # minimal prework for block 0's start
            qk_group(wq_sb, 0, q_sbs[0], 0, "scalar", ptag="s")()
            qk_group(wk_sb, 0, k_sbs[0], 0, "vector", ptag="s")()

            # deferred tasks: slot u -> emitters (run just before QK(u))
            tasks = {}

            def add_task(u, t):
                tasks.setdefault(u, []).append(t)

            add_task(0, v_sub(0))
            add_task(0, v_sub(1))
            for j in range(2, 16):
                add_task(j - 2, v_sub(j))
            add_task(1, qk_group(wk_sb, 0, k_sbs[0], 1))
            add_task(5, qk_group(wk_sb, 0, k_sbs[0], 2))
            add_task(9, qk_group(wk_sb, 0, k_sbs[0], 3))
            add_task(14, qk_group(wq_sb, 0, q_sbs[0], 1))
            add_task(20, qk_group(wq_sb, 0, q_sbs[0], 2))
            add_task(36, qk_group(wq_sb, 0, q_sbs[0], 3))
            add_task(46, qk_group(wk_sb, 1, k_sbs[1], 0))
            add_task(50, qk_group(wk_sb, 1, k_sbs[1], 1))
            add_task(54, qk_group(wk_sb, 1, k_sbs[1], 2))
            add_task(58, qk_group(wk_sb, 1, k_sbs[1], 3))
            add_task(60, qk_group(wq_sb, 1, q_sbs[1], 0))
            add_task(74, qk_group(wq_sb, 1, q_sbs[1], 1))
            add_task(90, qk_group(wq_sb, 1, q_sbs[1], 2))
            add_task(106, qk_group(wq_sb, 1, q_sbs[1], 3))

            # ---- attention main loop ----
            pending = []
            dma_rr = [0]

            def make_flush_unit(h, ib2, cc, o2T):
                def u():
                    tg = ib2 * 4 + cc
                    ps_t = ppool.tile([128, 512], MDT, tag="t")
                    nc.tensor.transpose(
                        ps_t[:, 0:128],
                        o2T[:, cc * 128 : (cc + 1) * 128],
                        ident[:, :],
                    )
                    rz = rzpool.tile([128, 1], FP, tag="rz")
                    nc.vector.reciprocal(out=rz, in_=ps_t[:, D : D + 1])
                    nc.vector.tensor_scalar_mul(
                        out=out2_sb[:, h, tg, :],
                        in0=ps_t[:, 0:D],
                        scalar1=rz,
                    )
                    # direct SBUF->SBUF scramble: out2[(pa pb), d] ->
                    # outrs rows (h%2)*64 + 4*tg + pa, cols pb*64 + d
                    r0 = (h % 2) * 64 + 4 * tg
                    weng = nc.sync if dma_rr[0] % 2 == 0 else nc.gpsimd
                    dma_rr[0] += 1
                    weng.dma_start(
                        out=outrs_sbs[h // 2][r0 : r0 + 4, :].rearrange(
                            "q (b d) -> q b d", d=D
                        ),
                        in_=out2_sb[:, h, tg, :],
                    )
                return u

            DRAIN_JTS = (2, 4, 6, 8, 10, 12, 14, 15)
            o_tiles = None
            for u in range(128):
                g, ib, jt = u // 64, (u // 16) % 4, u % 16
                i0 = ib * 512
                if jt == 0:
                    ps_oA = ppool.tile([128, 512], FP, tag="o")
                    ps_oB = ppool.tile([128, 512], FP, tag="o")
                    o_tiles = (ps_oA, ps_oB)
                ps_s = ppool.tile([128, 1024], FP, tag="s")
                for hp in range(2):
                    p0 = hp * 64
                    nc.tensor.matmul(
                        ps_s[:, hp * 512 : (hp + 1) * 512],
                        k_sbs[g][p0 : p0 + 64, jt * 128 : (jt + 1) * 128],
                        q_sbs[g][p0 : p0 + 64, i0 : i0 + 512],
                        start=True,
                        stop=True,
                    )
                ex = epool.tile([128, 1024], MDT, tag="exp")
                nc.scalar.activation(
                    out=ex, in_=ps_s, func=mybir.ActivationFunctionType.Exp
                )
                if jt in DRAIN_JTS and pending:
                    pending.pop(0)()
                for t in tasks.get(u, ()):
                    t()
                for hp in range(2):
                    nc.tensor.matmul(
                        o_tiles[hp][0 : D + 1, :],
                        vT1s[jt // 4][:, jt % 4, 2 * g + hp, :],
                        ex[:, hp * 512 : (hp + 1) * 512],
                        start=(jt == 0),
                        stop=(jt == NJ - 1),
                    )
                if jt == NJ - 1:
                    # free the PSUM accumulators promptly, then queue flush
                    for hp in range(2):
                        h = 2 * g + hp
                        o2T = o2pool.tile([128, 512], MDT, tag="o2T")
                        nc.vector.tensor_copy(
                            out=o2T[0 : D + 1, :], in_=o_tiles[hp][0 : D + 1, :]
                        )
                        for cc in range(4):
                            pending.append(make_flush_unit(h, ib, cc, o2T))

            # ---- tail: drain last flushes interleaved with og0 rc0 MMs ----
            psA = ppool.tile([128, 1024], FP, tag="s")
            psB = ppool.tile([128, 1024], FP, tag="s")
            ps_slots = [
                psA[:, 0:512],
                psA[:, 512:1024],
                psB[:, 0:512],
                psB[:, 512:1024],
                None,  # filled with 't' tiles after the flush drain
                None,
            ]

            def phd_mm(it, rc, start, stop):
                og, lb = it // NIB, it % NIB
                nc.tensor.matmul(
                    ps_slots[it % 6],
                    wo_sb[:, rc, og * 128 : (og + 1) * 128],
                    outrs_sbs[rc][:, lb * 512 : (lb + 1) * 512],
                    start=start,
                    stop=stop,
                )

            # og0 rc0 interleaved with the last block's flush units
            fl_i = 0
            for it in range(NIB):  # og0, lb 0..3
                while fl_i < 2 * (it + 1) and pending:
                    pending.pop(0)()
                    fl_i += 1
                phd_mm(it, 0, start=True, stop=False)
            while pending:
                pending.pop(0)()
            # safe to claim the 't' rotation now that flushes are all emitted
            pst0 = ppool.tile([128, 512], FP, tag="t")
            pst1 = ppool.tile([128, 512], FP, tag="t")
            ps_slots[4] = pst0
            ps_slots[5] = pst1
            # finish og0 (rc1) then stream og1..og3; copies alternate engines
            fouts = [
                fpool.tile([128, L], MDT, tag="fout", name=f"fo{og}")
                for og in range(4)
            ]
            for it in range(NIB):
                phd_mm(it, 1, start=False, stop=True)
                fsl = fouts[0][:, (it % NIB) * 512 : (it % NIB + 1) * 512]
                if it % 2 == 0:
                    nc.vector.tensor_copy(out=fsl, in_=ps_slots[it % 6])
                else:
                    nc.scalar.copy(out=fsl, in_=ps_slots[it % 6])
            nc.sync.dma_start(
                out=out_ext[0:128, 0:1024], in_=fouts[0][:, 0:1024]
            )
            nc.gpsimd.dma_start(
                out=out_ext[0:128, 1024:2048], in_=fouts[0][:, 1024:2048]
            )
            for it in range(NIB, 4 * NIB):
                og, lb = it // NIB, it % NIB
                phd_mm(it, 0, start=True, stop=False)
                phd_mm(it, 1, start=False, stop=True)
                fsl = fouts[og][:, lb * 512 : (lb + 1) * 512]
                if it % 2 == 0:
                    nc.vector.tensor_copy(out=fsl, in_=ps_slots[it % 6])
                else:
                    nc.scalar.copy(out=fsl, in_=ps_slots[it % 6])
                if lb == NIB - 1:
                    r = og * 128
                    nc.sync.dma_start(
                        out=out_ext[r : r + 128, 0:1024],
                        in_=fouts[og][:, 0:1024],
                    )
                    nc.gpsimd.dma_start(
                        out=out_ext[r : r + 128, 1024:2048],
                        in_=fouts[og][:, 1024:2048],
                    )

    _split_waits(nc)
    return nc


def _get_nc():
    key = str(MM_DT)
    if key not in _NC_CACHE:
        _NC_CACHE[key] = build_nc()
    return _NC_CACHE[key]


def _prepack_x(xb):
    # x[b] (C, L) -> [p, lb, ci, n] with c = ci*128+p, l = lb*512+n
    return np.ascontiguousarray(
        xb.reshape(NCC, 128, NIB, 512).transpose(1, 2, 0, 3)
    )


def _prepack_w(w):
    # (C, 256) -> [p, ci, n]
    return np.ascontiguousarray(w.reshape(NCC, 128, HPC * D).transpose(1, 0, 2))


def _prepack_wo(w):
    # (256, C) -> [p, rc, o]
    return np.ascontiguousarray(w.reshape(2, 128, C).transpose(1, 0, 2))


def kernel(x, w_qkv, w_out, b_out):
    global LAST_RESULT
    x = np.asarray(x, dtype=np.float32)
    w_qkv = np.asarray(w_qkv, dtype=np.float32)
    w_out = np.asarray(w_out, dtype=np.float32)
    b_out = np.asarray(b_out, dtype=np.float32)

    scale = D**-0.5
    bf16 = ml_dtypes.bfloat16
    xs = [_prepack_x(x[b]).astype(bf16) for b in range(B)]
    in_maps = []
    for m in range(NCORES):
        b = m // 2
        hs = [4 * (m % 2) + i for i in range(HPC)]
        q_rows = np.concatenate([np.arange(h * D, (h + 1) * D) for h in hs])
        wq = np.ascontiguousarray((w_qkv[q_rows, :] * scale).T)
        wk = np.ascontiguousarray(w_qkv[C + q_rows, :].T)
        wv = np.ascontiguousarray(w_qkv[2 * C + q_rows, :].T)
        wo = np.ascontiguousarray(w_out[:, q_rows].T)
        in_maps.append(
            {
                "x": xs[b],
                "wq": _prepack_w(wq).astype(bf16),
                "wk": _prepack_w(wk).astype(bf16),
                "wv": _prepack_w(wv).astype(bf16),
                "wo": _prepack_wo(wo).astype(bf16),
            }
        )

    nc = _get_nc()
    res = run_bass_kernel_spmd(
        nc, in_maps, core_ids=list(range(NCORES)), trace=TRACE_MODE
    )
    LAST_RESULT = res

    out = np.empty((B, C, L), dtype=np.float32)
    for b in range(B):
        out[b] = res.results[2 * b]["out"].astype(np.float32) + res.results[
            2 * b + 1
        ]["out"].astype(np.float32)
        out[b] += b_out[:, None]
    return out
